# revision 1
# baseline (speedup 1.0000x reference)
"""Trainium2 Bass kernel for nn_Block_34256659153605 (dual-branch linear-attention
transformer block). Data-parallel over batch B=8 across 8 NeuronCores; each core
runs the full block for one batch item.

Device layout (per core):
  - Activations live CT ("channels-on-partitions"): X^T tiles [i][c] of shape
    (128, 512) = X^T[128i:128(i+1), 512c:512(c+1)].
  - kp/vp are NT (tokens-on-partitions) tiles (128 tok, 512 ch), so the
    token-softmax denominator is a PE ones-matmul column-sum and ctx = k^T v
    contracts tokens on the partition axis.
  - Matmul operands bf16 (fp32 PSUM accumulation); residual adds are folded
    into PSUM via identity-matmuls (float32r for the fp32 x/y inputs).
  - pos embeddings folded on host: (x+pos)@W = x@W + (pos@W), the latter
    precomputed in numpy and accumulated on-device via identity-matmul.
  - LayerNorm over channels: PE ones-matmul sums -> per-token stats rows ->
    GPSIMD partition_broadcast -> fused DVE/GPSIMD apply.
"""

import os
import sys
import numpy as np

if "/opt/trn_rl_repo" not in sys.path:
    sys.path.insert(0, "/opt/trn_rl_repo")

import ml_dtypes
from contextlib import ExitStack

import concourse.bass as bass
import concourse.mybir as mybir
import concourse.tile as tile
from concourse import bacc
from concourse.masks import make_identity

P = 128
C = 512
H = 4
HID = 4 * C
CT = C // P          # 4 channel blocks
HT = HID // P        # 16 hidden blocks
FD = 512             # token chunk size
EPS = 1e-5

bf16 = mybir.dt.bfloat16
f32 = mybir.dt.float32
f32r = mybir.dt.float32r
AF = mybir.ActivationFunctionType
ALU = mybir.AluOpType

ATTN_W = ["sa_q", "sa_k", "sa_v", "sa_r", "ca_q", "ca_k", "ca_v", "ca_r"]
BIAS_N = ["sa_q", "sa_k", "sa_v", "sa_r", "ca_q", "ca_k", "ca_v", "ca_r",
          "mlp1", "mlp2"]


def build_nc(N=2048, ln_affine=False, biases=frozenset()):
    NCH = N // FD
    nc = bacc.Bacc("TRN2", debug=False)

    dr = {}
    def din(name, shape, dt, kind="ExternalInput"):
        dr[name] = nc.dram_tensor(name, shape, dt, kind=kind).ap()

    din("xT_bf", (C, N), bf16)
    din("yT_bf", (C, N), bf16)
    din("qT_bf", (C, N), bf16)
    for w in ATTN_W:
        din(w + "_w", (C, C), bf16)
    din("mlp_w1", (C, HID), bf16)
    din("mlp_w2", (HID, C), bf16)
    for nm in ["pq_sa_x", "pq_ca_x", "pq_sa_y", "pq_ca_y"]:
        din(nm, (C, N), bf16)
    for nm in ["pk_sa_x", "pk_ca_x", "pk_sa_y", "pk_ca_y"]:
        din(nm, (N, C), bf16)
    for bn in biases:
        din("b_" + bn, (1, HID if bn == "mlp1" else C), bf16)
    if ln_affine:
        din("ln_g", (C,), f32)
        din("ln_b", (C,), f32)
    for nm in ["z_osa", "z_oca", "z_oo", "z_ysa", "z_yca"]:
        din(nm, (C, N), bf16, kind="Internal")
    for nm in ["xsa", "ysa", "xca", "yca", "xml", "yml"]:
        din("st_" + nm, (4, N), f32, kind="Internal")
    for nm in ["xsa", "ysa", "xca", "yca"]:
        din("rq_" + nm, (4, N), bf16, kind="Internal")
    out_d = nc.dram_tensor("yOT", (C, N), f32, kind="ExternalOutput").ap()

    with tile.TileContext(nc) as tc, ExitStack() as ctx:
        consts = ctx.enter_context(tc.tile_pool(name="consts", bufs=1))
        a16 = ctx.enter_context(tc.tile_pool(name="a16", bufs=2))
        a32 = ctx.enter_context(tc.tile_pool(name="a32", bufs=2))
        pmm = ctx.enter_context(tc.tile_pool(name="pmm", bufs=6, space="PSUM"))
        psm = ctx.enter_context(tc.tile_pool(name="psm", bufs=2, space="PSUM"))

        # ---------------- persistent constants ----------------
        def wload(name, dram, nblk, fd):
            t = consts.tile([P, nblk, fd], bf16, name=name)
            nc.sync.dma_start(out=t, in_=dram.rearrange("(i p) c -> p i c", p=P))
            return t

        wsb = {w: wload("w_" + w, dr[w + "_w"], CT, C) for w in ATTN_W}
        w1sb = wload("w_mlp1", dr["mlp_w1"], CT, HID)
        w2sb = wload("w_mlp2", dr["mlp_w2"], HT, C)

        id_bf = consts.tile([P, P], bf16, name="id_bf")
        make_identity(nc, id_bf)
        ones_bf = consts.tile([P, 1], bf16, name="ones_bf")
        nc.vector.memset(ones_bf, 1.0)
        ones_f = consts.tile([P, 1], f32, name="ones_f")
        nc.vector.memset(ones_f, 1.0)
        ones_row = consts.tile([1, FD], bf16, name="ones_row")
        nc.vector.memset(ones_row, 1.0)
        eps_t = consts.tile([P, 1], f32, name="eps_t")
        nc.vector.memset(eps_t, EPS)
        brow = {}
        for bn in biases:
            bt = consts.tile([1, HID if bn == "mlp1" else C], bf16, name="br_" + bn)
            nc.sync.dma_start(out=bt, in_=dr["b_" + bn])
            brow[bn] = bt
        if ln_affine:
            g_col = consts.tile([P, CT], f32, name="g_col")
            b_col = consts.tile([P, CT], f32, name="b_col")
            nc.sync.dma_start(out=g_col, in_=dr["ln_g"].rearrange("(i p) -> p i", p=P))
            nc.sync.dma_start(out=b_col, in_=dr["ln_b"].rearrange("(i p) -> p i", p=P))

        ct_view = lambda d: d.rearrange("(i p) n -> i p n", p=P)
        nt_view = lambda d: d.rearrange("(t p) c -> t p c", p=P)

        def load_ct_chunk(d, c, name, dt=bf16):
            v = ct_view(d)
            pool, tg = (a16, "ld16") if dt == bf16 else (a32, "ld32")
            out = []
            for i in range(CT):
                tl = pool.tile([P, FD], dt, name=name, tag=tg, bufs=6)
                nc.sync.dma_start(out=tl, in_=v[i, :, c * FD:(c + 1) * FD])
                out.append(tl)
            return out

        def bias_ct(ps, bn, blk):
            """psum (out-block blk, tok) += bias[128*blk:...] x ones_row"""
            nc.tensor.matmul(ps, lhsT=brow[bn][0:1, blk * P:(blk + 1) * P],
                             rhs=ones_row, start=False, stop=True)

        def bias_nt(ps, bn):
            """psum (tok, cout) += ones x bias_row"""
            nc.tensor.matmul(ps, lhsT=ones_row[0:1, 0:P], rhs=brow[bn],
                             start=False, stop=True)

        # ---------------- layernorm (over channels) ----------------
        # Incremental: stats matmuls are emitted per chunk right after the
        # chunk's h tiles are produced; rows+apply emitted per 2-chunk batch
        # so h tiles free early (bounds the h32 pool, avoids slot deadlock).
        def dram_bcast_row(a):
            """DRAM AP (1, F) -> broadcast AP (128, F)."""
            return bass.AP(tensor=a.tensor, offset=a.offset,
                           ap=[[0, P]] + [list(d) for d in a.ap[1:]])

        class LNState:
            def __init__(self, tag, zout_dr, final_f32):
                self.tag = tag
                self.zout = zout_dr
                self.final = final_f32
                self.s_ps = psm.tile([P, FD], f32, name=tag + "_sps", tag="sm")
                self.q_ps = psm.tile([P, FD], f32, name=tag + "_qps", tag="sm")
                self.rows = a32.tile([P, 2 * FD], f32, name=tag + "_rows",
                                     tag="rows32", bufs=2)
                self.stdr = dr["st_" + tag]
                self.hf = {}

        def ln_chunk(st, hf_c, hsq_c, c, hb_c):
            """Emit LN stats for chunk c (hf_c/hsq_c: lists over i); on batch
            boundaries do row math (via DRAM re-pack) + bcast + apply + DMA."""
            tag = st.tag
            for i in range(CT):
                nc.tensor.matmul(st.s_ps[32 * c:32 * c + 1, :],
                                 lhsT=ones_bf, rhs=hb_c[i],
                                 start=(i == 0), stop=(i == CT - 1),
                                 tile_position=(0, 32 * c))
            for i in range(CT):
                nc.tensor.matmul(st.q_ps[32 * c:32 * c + 1, :], lhsT=ones_bf,
                                 rhs=hsq_c[i], start=(i == 0),
                                 stop=(i == CT - 1), tile_position=(0, 32 * c))
            # evict this chunk's stat rows and stage them to DRAM (rows live
            # at partition 32c; single-partition APs are legal everywhere)
            r_ = st.rows
            nc.vector.tensor_copy(out=r_[32 * c:32 * c + 1, 0:FD],
                                  in_=st.s_ps[32 * c:32 * c + 1, :])
            nc.vector.tensor_copy(out=r_[32 * c:32 * c + 1, FD:2 * FD],
                                  in_=st.q_ps[32 * c:32 * c + 1, :])
            nc.sync.dma_start(out=st.stdr[0, c * FD:(c + 1) * FD],
                              in_=r_[32 * c:32 * c + 1, 0:FD])
            nc.sync.dma_start(out=st.stdr[1, c * FD:(c + 1) * FD],
                              in_=r_[32 * c:32 * c + 1, FD:2 * FD])
            st.hf[c] = hf_c
            if c % 2 == 0 and c + 1 < NCH:
                return
            c0 = c - 1 if c % 2 == 1 else c
            nb = (c - c0 + 1) * (FD // P)       # packed cols for this batch
            j0 = c0 * (FD // P)
            # re-pack batch rows (tok-major) into (128, nb) via DRAM
            pk = a32.tile([P, 8, 3], f32, name=tag + "_pk", tag="snt", bufs=2)
            pv = lambda row: st.stdr[row, c0 * FD:c0 * FD + nb * P].rearrange(
                "(j p) -> p j", p=P)
            nc.sync.dma_start(out=pk[:, 0:nb, 0], in_=pv(0))
            nc.sync.dma_start(out=pk[:, 0:nb, 1], in_=pv(1))
            m_, q_, t_ = pk[:, 0:nb, 0], pk[:, 0:nb, 1], pk[:, 0:nb, 2]
            nc.vector.tensor_scalar_mul(out=m_, in0=m_, scalar1=1.0 / C)
            nc.vector.tensor_scalar_mul(out=q_, in0=q_, scalar1=1.0 / C)
            nc.vector.tensor_mul(out=t_, in0=m_, in1=m_)
            nc.vector.tensor_sub(out=t_, in0=q_, in1=t_)
            nc.scalar.activation(out=t_, in_=t_, func=AF.Sqrt,
                                 bias=eps_t[:, 0:1], scale=1.0)
            nc.vector.reciprocal(out=t_, in_=t_)           # rstd
            nc.vector.tensor_mul(out=m_, in0=m_, in1=t_)   # m*rstd
            nc.sync.dma_start(out=st.stdr[2, c0 * FD:c0 * FD + nb * P]
                              .rearrange("(j p) -> p j", p=P), in_=t_)
            nc.sync.dma_start(out=st.stdr[3, c0 * FD:c0 * FD + nb * P]
                              .rearrange("(j p) -> p j", p=P), in_=m_)
            zv = ct_view(st.zout)
            for cc in range(c0, c + 1):
                rb = a32.tile([P, FD], f32, name=tag + "_rstdb", tag="bc32", bufs=4)
                mb = a32.tile([P, FD], f32, name=tag + "_mb", tag="bc32", bufs=4)
                nc.sync.dma_start(out=rb, in_=dram_bcast_row(
                    st.stdr[2:3, cc * FD:(cc + 1) * FD]))
                nc.sync.dma_start(out=mb, in_=dram_bcast_row(
                    st.stdr[3:4, cc * FD:(cc + 1) * FD]))
                hb = st.hf.pop(cc)
                for i in range(CT):
                    nc.vector.tensor_mul(out=hb[i], in0=hb[i], in1=rb)
                    if st.final and not ln_affine:
                        zt = a32.tile([P, FD], f32, name=tag + "_zf",
                                      tag="zf32", bufs=3)
                    else:
                        zt = a16.tile([P, FD], bf16, name=tag + "_z",
                                      tag="z16", bufs=5)
                    nc.gpsimd.tensor_sub(out=zt, in0=hb[i], in1=mb)
                    if ln_affine:
                        if st.final:
                            z2 = a32.tile([P, FD], f32, name=tag + "_z2",
                                          tag="zf32", bufs=3)
                        else:
                            z2 = a16.tile([P, FD], bf16, name=tag + "_z2",
                                          tag="z16", bufs=5)
                        nc.vector.tensor_scalar(out=z2, in0=zt,
                                                scalar1=g_col[:, i:i + 1],
                                                scalar2=b_col[:, i:i + 1],
                                                op0=ALU.mult, op1=ALU.add)
                        zt = z2
                    nc.sync.dma_start(out=zv[i, :, cc * FD:(cc + 1) * FD], in_=zt)

        def evict_h(tag, ps):
            h_ = a32.tile([P, FD], f32, name=tag + "_h", tag="h32", bufs=12)
            nc.vector.tensor_copy(out=h_, in_=ps)
            sq = a16.tile([P, FD], bf16, name=tag + "_hsq", tag="sq16", bufs=6)
            nc.scalar.activation(out=sq, in_=ps, func=AF.Square)
            hb = a16.tile([P, FD], bf16, name=tag + "_hb", tag="hb16", bufs=6)
            nc.gpsimd.tensor_copy(out=hb, in_=h_)
            return h_, sq, hb

        def resid_mm(ps, rt, resid_f32, stop):
            nc.tensor.matmul(ps, lhsT=id_bf, rhs=rt, start=False, stop=stop)

        # ---------------- efficient attention ----------------
        def eattn(tag, qin, kin, W, posq, posk, resid, resid_f32, zout,
                  final_f32=False, bq=None, bk=None, bv=None, br=None):
            wq, wk, wv, wr = (wsb[W + "_q"], wsb[W + "_k"],
                              wsb[W + "_v"], wsb[W + "_r"])
            # ---- phase 1: kp / vp / s_k / ctx, token-tile ordered ----
            ctx_ps = [pmm.tile([P, P], f32, name=tag + "_ctx%d" % h, tag="mm")
                      for h in range(H)]
            sk_ps = psm.tile([P, FD], f32, name=tag + "_sk", tag="sm")
            pkv = nt_view(dr[posk]) if posk else None
            for c in range(NCH):
                kint = load_ct_chunk(dr[kin], c, tag + "_kin")
                for tt in range(4):
                    t = 4 * c + tt
                    kp = pmm.tile([P, FD], f32, name=tag + "_kp", tag="mm")
                    for i in range(CT):
                        nc.tensor.matmul(kp, lhsT=kint[i][:, tt * P:(tt + 1) * P],
                                         rhs=wk[:, i, :], start=(i == 0), stop=False)
                    if posk:
                        pk = a16.tile([P, FD], bf16, name=tag + "_pk", tag="pk16", bufs=3)
                        nc.sync.dma_start(out=pk, in_=pkv[t])
                        nc.tensor.matmul(kp, lhsT=id_bf, rhs=pk, start=False,
                                         stop=(bk is None))
                    if bk is not None:
                        bias_nt(kp, bk)
                    ek = a16.tile([P, FD], bf16, name=tag + "_ek", tag="kv16", bufs=5)
                    nc.scalar.activation(out=ek, in_=kp, func=AF.Exp)
                    vp = pmm.tile([P, FD], f32, name=tag + "_vp", tag="mm")
                    for i in range(CT):
                        nc.tensor.matmul(vp, lhsT=kint[i][:, tt * P:(tt + 1) * P],
                                         rhs=wv[:, i, :], start=(i == 0),
                                         stop=(i == CT - 1 and bv is None))
                    if bv is not None:
                        bias_nt(vp, bv)
                    vt = a16.tile([P, FD], bf16, name=tag + "_vt", tag="kv16", bufs=5)
                    nc.vector.tensor_copy(out=vt, in_=vp)
                    nc.tensor.matmul(sk_ps[0:1, :], lhsT=ones_bf, rhs=ek,
                                     start=(t == 0), stop=(t == 4 * NCH - 1))
                    for h in range(H):
                        nc.tensor.matmul(ctx_ps[h],
                                         lhsT=ek[:, h * P:(h + 1) * P],
                                         rhs=vt[:, h * P:(h + 1) * P],
                                         start=(t == 0), stop=(t == 4 * NCH - 1))
            # ---- phase 2: 1/s_k as columns; normalize ctx rows ----
            skrow = a32.tile([P, FD], f32, name=tag + "_skrow", tag="skrow", bufs=2)
            nc.vector.tensor_copy(out=skrow[0:1, :], in_=sk_ps[0:1, :])
            ktp = pmm.tile([P, H], f32, name=tag + "_ktp", tag="mm")
            for h in range(H):
                nc.tensor.transpose(ktp[:, h:h + 1], skrow[0:1, h * P:(h + 1) * P],
                                    ones_f[0:1, 0:1])
            rk = a32.tile([P, H], f32, name=tag + "_rk", tag="rk", bufs=2)
            nc.vector.reciprocal(out=rk, in_=ktp)
            ctx_bf = []
            for h in range(H):
                cb = a16.tile([P, P], bf16, name=tag + "_cbf", tag="cbf", bufs=8)
                nc.vector.tensor_scalar_mul(out=cb, in0=ctx_ps[h],
                                            scalar1=rk[:, h:h + 1])
                ctx_bf.append(cb)
            # ---- phase 3: qp / q-softmax / att / reproj+resid, chunk ordered ----
            pqv = ct_view(dr[posq]) if posq else None
            rv = ct_view(resid)
            lst = LNState(tag, zout, final_f32)
            for c in range(NCH):
                qint = load_ct_chunk(dr[qin], c, tag + "_qin")
                sq_ps = pmm.tile([P, FD], f32, name=tag + "_sq", tag="mm")
                eq = []
                for m in range(CT):
                    ps = pmm.tile([P, FD], f32, name=tag + "_qp", tag="mm")
                    for i in range(CT):
                        nc.tensor.matmul(ps, lhsT=wq[:, i, m * P:(m + 1) * P],
                                         rhs=qint[i], start=(i == 0),
                                         stop=(i == CT - 1 and posq is None
                                               and bq is None))
                    if posq:
                        pq = a16.tile([P, FD], bf16, name=tag + "_pq", tag="pk16", bufs=3)
                        nc.sync.dma_start(out=pq, in_=pqv[m, :, c * FD:(c + 1) * FD])
                        nc.tensor.matmul(ps, lhsT=id_bf, rhs=pq, start=False,
                                         stop=(bq is None))
                    if bq is not None:
                        bias_ct(ps, bq, m)
                    e = a16.tile([P, FD], bf16, name=tag + "_eq", tag="eq16", bufs=6)
                    nc.scalar.activation(out=e, in_=ps, func=AF.Exp)
                    eq.append(e)
                    nc.tensor.matmul(sq_ps[32 * m:32 * m + 1, :], lhsT=ones_bf,
                                     rhs=e, start=True, stop=True,
                                     tile_position=(0, 32 * m))
                rqr = a16.tile([P, FD], bf16, name=tag + "_rqr", tag="rqr", bufs=2)
                with nc.allow_low_precision(reason="bf16 softmax recip ok"):
                    for h in range(H):
                        nc.vector.reciprocal(out=rqr[32 * h:32 * h + 1, :],
                                             in_=sq_ps[32 * h:32 * h + 1, :])
                rq_dr = dr["rq_" + tag]
                for h in range(H):
                    nc.sync.dma_start(out=rq_dr[h, c * FD:(c + 1) * FD],
                                      in_=rqr[32 * h:32 * h + 1, :])
                for h in range(H):
                    rqb = a16.tile([P, FD], bf16, name=tag + "_rqb", tag="bc16", bufs=6)
                    nc.sync.dma_start(out=rqb, in_=dram_bcast_row(
                        rq_dr[h:h + 1, c * FD:(c + 1) * FD]))
                    aps = pmm.tile([P, FD], f32, name=tag + "_aps", tag="mm")
                    nc.tensor.matmul(aps, lhsT=ctx_bf[h], rhs=eq[h],
                                     start=True, stop=True)
                    ab = a16.tile([P, FD], bf16, name=tag + "_ab", tag="att16", bufs=6)
                    nc.vector.tensor_mul(out=ab, in0=aps, in1=rqb)
                    eq[h] = None
                    eq.append(ab)  # keep refs ordered: att tiles at eq[H+h]
                att = eq[H:]
                rts = []
                for i in range(CT):
                    rt = a16.tile([P, FD], bf16, name=tag + "_rt", tag="ld16", bufs=6)
                    nc.sync.dma_start(out=rt, in_=rv[i, :, c * FD:(c + 1) * FD])
                    rts.append(rt)
                hfc, hsqc, hbc = [], [], []
                for i in range(CT):
                    ps = pmm.tile([P, FD], f32, name=tag + "_rp", tag="mm")
                    for hh in range(CT):
                        nc.tensor.matmul(ps, lhsT=wr[:, hh, i * P:(i + 1) * P],
                                         rhs=att[hh], start=(hh == 0), stop=False)
                    resid_mm(ps, rts[i], resid_f32, stop=(br is None))
                    if br is not None:
                        bias_ct(ps, br, i)
                    h_, sq_, hb_ = evict_h(tag, ps)
                    hfc.append(h_)
                    hsqc.append(sq_)
                    hbc.append(hb_)
                ln_chunk(lst, hfc, hsqc, c, hbc)

        # ---------------- MLP ----------------
        def mlp(tag, zin, zout, final_f32, b1=None, b2=None):
            rv = ct_view(dr[zin])
            lst = LNState(tag, zout, final_f32)
            for c in range(NCH):
                zint = load_ct_chunk(dr[zin], c, tag + "_zin")
                u = []
                for ht in range(HT):
                    ps = pmm.tile([P, FD], f32, name=tag + "_f1", tag="mm")
                    for i in range(CT):
                        nc.tensor.matmul(ps, lhsT=w1sb[:, i, ht * P:(ht + 1) * P],
                                         rhs=zint[i], start=(i == 0),
                                         stop=(i == CT - 1 and b1 is None))
                    if b1 is not None:
                        bias_ct(ps, b1, ht)
                    ut = a16.tile([P, FD], bf16, name=tag + "_u", tag="u16", bufs=20)
                    if ht % 2 == 0:
                        nc.scalar.activation(out=ut, in_=ps, func=AF.Relu)
                    else:
                        nc.vector.tensor_scalar_max(out=ut, in0=ps, scalar1=0.0)
                    u.append(ut)
                rts = []
                for i in range(CT):
                    rt = a16.tile([P, FD], bf16, name=tag + "_rt", tag="ld16", bufs=6)
                    nc.sync.dma_start(out=rt, in_=rv[i, :, c * FD:(c + 1) * FD])
                    rts.append(rt)
                hfc, hsqc, hbc = [], [], []
                for i in range(CT):
                    ps = pmm.tile([P, FD], f32, name=tag + "_f2", tag="mm")
                    for ht in range(HT):
                        nc.tensor.matmul(ps, lhsT=w2sb[:, ht, i * P:(i + 1) * P],
                                         rhs=u[ht], start=(ht == 0), stop=False)
                    resid_mm(ps, rts[i], False, stop=(b2 is None))
                    if b2 is not None:
                        bias_ct(ps, b2, i)
                    h_, sq_, hb_ = evict_h(tag, ps)
                    hfc.append(h_)
                    hsqc.append(sq_)
                    hbc.append(hb_)
                ln_chunk(lst, hfc, hsqc, c, hbc)

        bb = lambda n: (n if n in biases else None)

        eattn("xsa", "xT_bf", "xT_bf", "sa", "pq_sa_x", "pk_sa_x",
              dr["xT_bf"], False, dr["z_osa"],
              bq=bb("sa_q"), bk=bb("sa_k"), bv=bb("sa_v"), br=bb("sa_r"))
        eattn("ysa", "yT_bf", "yT_bf", "sa", "pq_sa_y", "pk_sa_y",
              dr["yT_bf"], False, dr["z_ysa"],
              bq=bb("sa_q"), bk=bb("sa_k"), bv=bb("sa_v"), br=bb("sa_r"))
        eattn("xca", "qT_bf", "z_osa", "ca", "pq_ca_x", "pk_ca_x",
              dr["z_osa"], False, dr["z_oca"],
              bq=bb("ca_q"), bk=bb("ca_k"), bv=bb("ca_v"), br=bb("ca_r"))
        mlp("xml", "z_oca", dr["z_oo"], False, b1=bb("mlp1"), b2=bb("mlp2"))
        eattn("yca", "z_oo", "z_ysa", "ca", "pq_ca_y", "pk_ca_y",
              dr["z_ysa"], False, dr["z_yca"],
              bq=bb("ca_q"), bk=bb("ca_k"), bv=bb("ca_v"), br=bb("ca_r"))
        mlp("yml", "z_yca", out_d, True, b1=bb("mlp1"), b2=bb("mlp2"))

    nc.compile()
    return nc


# ======================= host side =======================

_NC_CACHE = {}
LAST_RESULT = None


def _get_nc(N, ln_affine, biases):
    key = (N, ln_affine, tuple(sorted(biases)))
    if key not in _NC_CACHE:
        _NC_CACHE[key] = build_nc(N, ln_affine, frozenset(biases))
    return _NC_CACHE[key]


def _bf(a):
    return np.ascontiguousarray(a.astype(ml_dtypes.bfloat16))


def host_prep(inputs, N):
    """Common (core-independent) in_map entries."""
    ws = {w: np.asarray(inputs[w + "_w"], np.float32) for w in ATTN_W}
    posx = np.asarray(inputs["pos_x"], np.float32)[0]  # (N, C)
    posy = np.asarray(inputs["pos_y"], np.float32)[0]
    m = {}
    for w in ATTN_W:
        m[w + "_w"] = _bf(ws[w])
    m["mlp_w1"] = _bf(np.asarray(inputs["mlp_w1"], np.float32))
    m["mlp_w2"] = _bf(np.asarray(inputs["mlp_w2"], np.float32))
    m["pq_sa_x"] = _bf((posx @ ws["sa_q"]).T)
    m["pq_ca_x"] = _bf((posx @ ws["ca_q"]).T)
    m["pq_sa_y"] = _bf((posy @ ws["sa_q"]).T)
    m["pq_ca_y"] = _bf((posy @ ws["ca_q"]).T)
    m["pk_sa_x"] = _bf(posx @ ws["sa_k"])
    m["pk_ca_x"] = _bf(posx @ ws["ca_k"])
    m["pk_sa_y"] = _bf(posy @ ws["sa_k"])
    m["pk_ca_y"] = _bf(posy @ ws["ca_k"])
    bias_arr = {"sa_q": "sa_q_b", "sa_k": "sa_k_b", "sa_v": "sa_v_b",
                "sa_r": "sa_r_b", "ca_q": "ca_q_b", "ca_k": "ca_k_b",
                "ca_v": "ca_v_b", "ca_r": "ca_r_b",
                "mlp1": "mlp_b1", "mlp2": "mlp_b2"}
    biases = set()
    for bn, an in bias_arr.items():
        arr = np.asarray(inputs[an], np.float32)
        if np.any(arr != 0):
            biases.add(bn)
            m["b_" + bn] = _bf(arr.reshape(1, -1))
    g = np.asarray(inputs["ln_g"], np.float32)
    b = np.asarray(inputs["ln_b"], np.float32)
    ln_affine = bool(np.any(g != 1) or np.any(b != 0))
    if ln_affine:
        m["ln_g"] = np.ascontiguousarray(g)
        m["ln_b"] = np.ascontiguousarray(b)
    return m, biases, ln_affine


def core_inputs(inputs, b):
    x = np.asarray(inputs["x"], np.float32)[b]
    y = np.asarray(inputs["y"], np.float32)[b]
    q = np.asarray(inputs["q"], np.float32)[b]
    return {"xT_bf": _bf(x.T), "yT_bf": _bf(y.T), "qT_bf": _bf(q.T)}


def kernel(**inputs):
    from concourse import bass_utils
    N = np.asarray(inputs["x"]).shape[1]
    B = np.asarray(inputs["x"]).shape[0]
    common, biases, ln_affine = host_prep(inputs, N)
    nc = _get_nc(N, ln_affine, biases)
    in_maps = []
    for b in range(B):
        m = dict(common)
        m.update(core_inputs(inputs, b))
        in_maps.append(m)
    res = bass_utils.run_bass_kernel_spmd(nc, in_maps, core_ids=list(range(B)))
    global LAST_RESULT
    LAST_RESULT = res
    out = np.stack([r["yOT"].T for r in res.results], axis=0)
    return np.ascontiguousarray(out.astype(np.float32))



# revision 4
# speedup vs baseline: 1.2675x; 1.2675x over previous
"""Trainium2 Bass kernel for nn_Block_34256659153605 (dual-branch linear-attention
transformer block). Data-parallel over batch B=8 across 8 NeuronCores; each core
runs the full block for one batch item.

Layout strategy (v2, token-major):
  - Projections produce NT tiles (128 tokens on partitions, channels free), so
    every per-token reduction (q-softmax denominators, LayerNorm mean/var) is a
    partition-parallel free-axis accumulate (ACT/DVE accum_out), with small
    [128,H] reciprocals -- no single-partition DVE ops, no DRAM stat round
    trips, no partition broadcasts.
  - The k-softmax (over tokens) stays a PE ones-matmul; its per-channel
    reciprocal is folded into M_h = (1/s_k) * ctx_h @ Wr_h, which also folds
    the context matrix into the reprojection (no separate att matmul).
  - Next layer's contractions need channel-major (CT) operands, produced by
    PE transposes (128-col matmuls against identity) of the LN output tiles.
  - Residual adds and positional-projection adds ride the PSUM evictions on
    DVE (scalar_tensor_tensor) instead of PE identity-matmuls.
  - pos embeddings folded on host: (x+pos)@W = x@W + (pos@W + b), the latter
    precomputed in numpy (q/k biases folded there too).
"""

import os
import sys
import numpy as np

if "/opt/trn_rl_repo" not in sys.path:
    sys.path.insert(0, "/opt/trn_rl_repo")

import ml_dtypes
from contextlib import ExitStack

import concourse.bass as bass
import concourse.mybir as mybir
import concourse.tile as tile
from concourse import bacc
from concourse.masks import make_identity

P = 128
C = 512
H = 4
HID = 4 * C
CT = C // P          # 4 channel blocks
HT = HID // P        # 16 hidden blocks
FD = 512             # token chunk size
EPS = 1e-5

bf16 = mybir.dt.bfloat16
f32 = mybir.dt.float32
AF = mybir.ActivationFunctionType
ALU = mybir.AluOpType

ATTN_W = ["sa_q", "sa_k", "sa_v", "sa_r", "ca_q", "ca_k", "ca_v", "ca_r"]


def build_nc(N=2048, ln_affine=False, biases=frozenset()):
    NCH = N // FD
    NTL = N // P  # token tiles
    nc = bacc.Bacc("TRN2", debug=False)

    dr = {}
    def din(name, shape, dt, kind="ExternalInput"):
        dr[name] = nc.dram_tensor(name, shape, dt, kind=kind).ap()

    din("xT_bf", (C, N), bf16)
    din("yT_bf", (C, N), bf16)
    din("qT_bf", (C, N), bf16)
    din("x_nt", (N, C), bf16)
    din("y_nt", (N, C), bf16)
    for w in ATTN_W:
        din(w + "_w", (C, C), bf16)
    din("mlp_w1", (C, HID), bf16)
    din("mlp_w2", (HID, C), bf16)
    # pos projections (+ folded q/k biases), token-major (N, C)
    for nm in ["pq_sa_x", "pq_ca_x", "pq_sa_y", "pq_ca_y",
               "pk_sa_x", "pk_ca_x", "pk_sa_y", "pk_ca_y"]:
        din(nm, (N, C), bf16)
    for bn in biases:
        if bn == "mlp1":
            din("b_mlp1", (HID,), f32)
        else:
            din("b_" + bn, (1, C), bf16)
    if ln_affine:
        din("ln_g", (1, C), f32)
        din("ln_b", (1, C), f32)
    # inter-layer handoffs
    for nm in ["zT_osa", "zT_oca", "zT_oo", "zT_ysa", "zT_yca"]:
        din(nm, (C, N), bf16, kind="Internal")
    for nm in ["zn_osa", "zn_oca", "zn_ysa", "zn_yca"]:
        din(nm, (N, C), bf16, kind="Internal")
    out_d = nc.dram_tensor("yO", (N, C), f32, kind="ExternalOutput").ap()

    ctv = lambda d: d.rearrange("(i p) n -> p i n", p=P)   # chunk loads
    ctw = lambda d: d.rearrange("(i p) n -> p i n", p=P)   # zT writes
    ntv = lambda d: d.rearrange("(t p) c -> t p c", p=P)   # NT tiles

    with tile.TileContext(nc) as tc, ExitStack() as ctx:
        consts = ctx.enter_context(tc.tile_pool(name="consts", bufs=1))
        a16 = ctx.enter_context(tc.tile_pool(name="a16", bufs=2))
        a32 = ctx.enter_context(tc.tile_pool(name="a32", bufs=2))
        pmm = ctx.enter_context(tc.tile_pool(name="pmm", bufs=4, space="PSUM"))
        psm = ctx.enter_context(tc.tile_pool(name="psm", bufs=2, space="PSUM"))
        ptp = ctx.enter_context(tc.tile_pool(name="ptp", bufs=2, space="PSUM"))

        # ---------------- persistent constants ----------------
        def wload(name, dram, nblk, fd):
            t = consts.tile([P, nblk, fd], bf16, name=name)
            nc.sync.dma_start(out=t, in_=dram.rearrange("(i p) c -> p i c", p=P))
            return t

        wsb = {w: wload("w_" + w, dr[w + "_w"], CT, C) for w in ATTN_W}
        w1sb = wload("w_mlp1", dr["mlp_w1"], CT, HID)
        w2sb = wload("w_mlp2", dr["mlp_w2"], HT, C)

        id_bf = consts.tile([P, P], bf16, name="id_bf")
        make_identity(nc, id_bf)
        ones_bf = consts.tile([P, 1], bf16, name="ones_bf")
        nc.vector.memset(ones_bf, 1.0)
        ones_f = consts.tile([P, 1], f32, name="ones_f")
        nc.vector.memset(ones_f, 1.0)
        eps_t = consts.tile([P, 1], f32, name="eps_t")
        nc.vector.memset(eps_t, EPS)

        # bias / affine constants (only when active; zero in practice)
        bcast = {}

        def dram_bcast_row(a):
            """DRAM AP (1, F) -> broadcast AP (128, F)."""
            return bass.AP(tensor=a.tensor, offset=a.offset,
                           ap=[[0, P]] + [list(d) for d in a.ap[1:]])

        def row_bcast(name, src_row, dt):
            bt = consts.tile([P, C], dt, name=name + "_bc")
            nc.sync.dma_start(out=bt, in_=dram_bcast_row(src_row))
            return bt

        for bn in ("sa_v", "ca_v", "sa_r", "ca_r", "mlp2"):
            if bn in biases:
                bcast[bn] = row_bcast("b" + bn, dr["b_" + bn], bf16)
        b1c = None
        if "mlp1" in biases:
            b1c = consts.tile([P, HT], f32, name="b1c")
            nc.sync.dma_start(out=b1c,
                              in_=dr["b_mlp1"].rearrange("(i p) -> p i", p=P))
        g_bc = b_bc = None
        if ln_affine:
            g_bc = row_bcast("lng", dr["ln_g"], f32)
            b_bc = row_bcast("lnb", dr["ln_b"], f32)

        # ---------------- helpers ----------------
        def load_ct_chunk(d, c, name):
            tl = a16.tile([P, CT, FD], bf16, name=name, tag="ld16", bufs=3)
            nc.sync.dma_start(out=tl, in_=ctv(d)[:, :, c * FD:(c + 1) * FD])
            return tl

        def load_nt(d, t, name, tag="nt16", bufs=6):
            tl = a16.tile([P, C], bf16, name=name, tag=tag, bufs=bufs)
            nc.sync.dma_start(out=tl, in_=ntv(d)[t])
            return tl

        def ln_tile(tag, ps, rt, badd, final, zn_dr, zT_dr, zTc, t):
            """PSUM (tok, chan) + resid -> LN -> z tile (+ transpose to zTc)."""
            tt = t % 4
            st2 = a32.tile([P, 2], f32, name=tag + "_st2", tag="st2", bufs=8)
            h_ = a32.tile([P, FD], f32, name=tag + "_h", tag="h32", bufs=4)
            nc.vector.scalar_tensor_tensor(
                out=h_, in0=ps, scalar=1.0, in1=rt,
                op0=ALU.mult, op1=ALU.add, accum_out=st2[:, 0:1])
            if badd is not None:
                nc.vector.tensor_tensor(out=h_, in0=h_, in1=badd, op=ALU.add)
            hsq = a16.tile([P, FD], bf16, name=tag + "_hsq", tag="sq16", bufs=2)
            nc.scalar.activation(out=hsq, in_=h_, func=AF.Square,
                                 accum_out=st2[:, 1:2])
            mq = a32.tile([P, 2], f32, name=tag + "_mq", tag="st2", bufs=8)
            nc.vector.tensor_scalar_mul(out=mq, in0=st2, scalar1=1.0 / C)
            vv = a32.tile([P, 2], f32, name=tag + "_vv", tag="st2", bufs=8)
            m_, q_ = mq[:, 0:1], mq[:, 1:2]
            nc.vector.tensor_tensor(out=vv[:, 0:1], in0=m_, in1=m_, op=ALU.mult)
            nc.vector.tensor_tensor(out=vv[:, 1:2], in0=q_, in1=vv[:, 0:1],
                                    op=ALU.subtract)
            nc.scalar.activation(out=vv[:, 0:1], in_=vv[:, 1:2], func=AF.Sqrt,
                                 bias=eps_t[:, 0:1])
            rstd = a32.tile([P, 1], f32, name=tag + "_rstd", tag="st1", bufs=8)
            nc.vector.reciprocal(out=rstd, in_=vv[:, 0:1])
            if final:
                zf = a32.tile([P, FD], f32, name=tag + "_zf", tag="zf32", bufs=3)
                nc.vector.tensor_scalar(out=zf, in0=h_, scalar1=m_,
                                        scalar2=rstd, op0=ALU.subtract,
                                        op1=ALU.mult)
                nc.sync.dma_start(out=ntv(out_d)[t], in_=zf)
                return
            if ln_affine:
                z0 = a32.tile([P, FD], f32, name=tag + "_z0", tag="zf32", bufs=3)
                nc.vector.tensor_scalar(out=z0, in0=h_, scalar1=m_,
                                        scalar2=rstd, op0=ALU.subtract,
                                        op1=ALU.mult)
                z1 = a32.tile([P, FD], f32, name=tag + "_z1", tag="zf32", bufs=3)
                nc.vector.tensor_tensor(out=z1, in0=z0, in1=g_bc, op=ALU.mult)
                zb = a16.tile([P, FD], bf16, name=tag + "_zb", tag="zb16", bufs=3)
                nc.vector.tensor_tensor(out=zb, in0=z1, in1=b_bc, op=ALU.add)
            else:
                zb = a16.tile([P, FD], bf16, name=tag + "_zb", tag="zb16", bufs=3)
                nc.vector.tensor_scalar(out=zb, in0=h_, scalar1=m_,
                                        scalar2=rstd, op0=ALU.subtract,
                                        op1=ALU.mult)
            if zn_dr is not None:
                nc.sync.dma_start(out=ntv(zn_dr)[t], in_=zb)
            # transpose z tile -> CT copy for next layer's contractions
            tp = ptp.tile([P, CT, P], f32, name=tag + "_ztp", tag="tp")
            for i in range(CT):
                nc.tensor.matmul(tp[:, i, :], lhsT=zb[:, i * P:(i + 1) * P],
                                 rhs=id_bf, start=True, stop=True)
            nc.vector.tensor_copy(out=zTc[:, :, tt * P:(tt + 1) * P], in_=tp)
            if tt == 3:
                c = t // 4
                nc.sync.dma_start(
                    out=ctw(zT_dr)[:, :, c * FD:(c + 1) * FD], in_=zTc)

        def new_zTc(tag):
            return a16.tile([P, CT, FD], bf16, name=tag + "_zTc",
                            tag="zTc", bufs=2)

        # ---------------- efficient attention ----------------
        def eattn(tag, kinT, qinT, W, pk, pq, resid, zn_dr, zT_dr,
                  final=False, bv=None, br=None):
            wq, wk, wv, wr = (wsb[W + "_q"], wsb[W + "_k"],
                              wsb[W + "_v"], wsb[W + "_r"])
            with tc.spectator_scope(tag):
                # ---- phase 1: kp/vp/ek/vt, sk (ones-MM), ctx^T ----
                sk_ps = psm.tile([P, FD], f32, name=tag + "_sk", tag="sm")
                ctxT_ps = psm.tile([P, H, P], f32, name=tag + "_ctxT", tag="sm")
                for cch in range(NCH):
                    kin = load_ct_chunk(dr[kinT], cch, tag + "_kin")
                    for tsub in range(4):
                        t = 4 * cch + tsub
                        kp = pmm.tile([P, FD], f32, name=tag + "_kp", tag="mm")
                        for i in range(CT):
                            nc.tensor.matmul(
                                kp, lhsT=kin[:, i, tsub * P:(tsub + 1) * P],
                                rhs=wk[:, i, :], start=(i == 0),
                                stop=(i == CT - 1))
                        pkt = load_nt(dr[pk], t, tag + "_pkt")
                        kadd = a16.tile([P, FD], bf16, name=tag + "_kadd",
                                        tag="add16", bufs=3)
                        nc.vector.scalar_tensor_tensor(
                            out=kadd, in0=kp, scalar=1.0, in1=pkt,
                            op0=ALU.mult, op1=ALU.add)
                        ek = a16.tile([P, FD], bf16, name=tag + "_ek",
                                      tag="ek16", bufs=3)
                        nc.scalar.activation(out=ek, in_=kadd, func=AF.Exp)
                        vp = pmm.tile([P, FD], f32, name=tag + "_vp", tag="mm")
                        for i in range(CT):
                            nc.tensor.matmul(
                                vp, lhsT=kin[:, i, tsub * P:(tsub + 1) * P],
                                rhs=wv[:, i, :], start=(i == 0),
                                stop=(i == CT - 1))
                        vt = a16.tile([P, FD], bf16, name=tag + "_vt",
                                      tag="vt16", bufs=3)
                        if bv is not None:
                            nc.vector.tensor_tensor(out=vt, in0=vp, in1=bv,
                                                    op=ALU.add)
                        else:
                            nc.vector.tensor_copy(out=vt, in_=vp)
                        nc.tensor.matmul(sk_ps[0:1, :], lhsT=ones_bf, rhs=ek,
                                         start=(t == 0), stop=(t == NTL - 1))
                        for h in range(H):
                            nc.tensor.matmul(
                                ctxT_ps[:, h, :],
                                lhsT=vt[:, h * P:(h + 1) * P],
                                rhs=ek[:, h * P:(h + 1) * P],
                                start=(t == 0), stop=(t == NTL - 1))
                # ---- phase 2: rk = 1/s_k ; M_h = rk * ctx_h @ Wr_h ----
                skrow = a32.tile([1, FD], f32, name=tag + "_skrow",
                                 tag="skrow", bufs=2)
                nc.vector.tensor_copy(out=skrow, in_=sk_ps[0:1, :])
                ktp = ptp.tile([P, H], f32, name=tag + "_ktp", tag="tp")
                for h in range(H):
                    nc.tensor.transpose(ktp[:, h:h + 1],
                                        skrow[0:1, h * P:(h + 1) * P],
                                        ones_f[0:1, 0:1])
                rk = a32.tile([P, H], f32, name=tag + "_rk", tag="rk", bufs=4)
                nc.vector.reciprocal(out=rk, in_=ktp)
                ctxT_bf = a16.tile([P, H, P], bf16, name=tag + "_ctxbf",
                                   tag="ctx16", bufs=2)
                nc.vector.tensor_copy(out=ctxT_bf, in_=ctxT_ps)
                M_bf = []
                for h in range(H):
                    mps = pmm.tile([P, FD], f32, name=tag + "_mps", tag="mm")
                    nc.tensor.matmul(mps, lhsT=ctxT_bf[:, h, :],
                                     rhs=wr[:, h, :], start=True, stop=True)
                    mb = a16.tile([P, FD], bf16, name=tag + "_mbf",
                                  tag="M16", bufs=5)
                    nc.vector.tensor_scalar_mul(out=mb, in0=mps,
                                                scalar1=rk[:, h:h + 1])
                    M_bf.append(mb)
                # ---- phase 3: qp -> softmax -> seq^T -> out_nt -> LN ----
                zTc = None
                for cch in range(NCH):
                    qin = load_ct_chunk(dr[qinT], cch, tag + "_qin")
                    if not final:
                        zTc = new_zTc(tag)
                    for tsub in range(4):
                        t = 4 * cch + tsub
                        qp = pmm.tile([P, FD], f32, name=tag + "_qp", tag="mm")
                        for i in range(CT):
                            nc.tensor.matmul(
                                qp, lhsT=qin[:, i, tsub * P:(tsub + 1) * P],
                                rhs=wq[:, i, :], start=(i == 0),
                                stop=(i == CT - 1))
                        pqt = load_nt(dr[pq], t, tag + "_pqt")
                        qadd = a16.tile([P, FD], bf16, name=tag + "_qadd",
                                        tag="add16", bufs=3)
                        nc.vector.scalar_tensor_tensor(
                            out=qadd, in0=qp, scalar=1.0, in1=pqt,
                            op0=ALU.mult, op1=ALU.add)
                        sq4 = a32.tile([P, H], f32, name=tag + "_sq4",
                                       tag="rk", bufs=4)
                        eq = a16.tile([P, FD], bf16, name=tag + "_eq",
                                      tag="eq16", bufs=3)
                        for h in range(H):
                            nc.scalar.activation(
                                out=eq[:, h * P:(h + 1) * P],
                                in_=qadd[:, h * P:(h + 1) * P], func=AF.Exp,
                                accum_out=sq4[:, h:h + 1])
                        rq4 = a32.tile([P, H], f32, name=tag + "_rq4",
                                       tag="rk", bufs=4)
                        nc.vector.reciprocal(out=rq4, in_=sq4)
                        seq = a16.tile([P, FD], bf16, name=tag + "_seq",
                                       tag="seq16", bufs=3)
                        for h in range(H):
                            nc.vector.tensor_scalar_mul(
                                out=seq[:, h * P:(h + 1) * P],
                                in0=eq[:, h * P:(h + 1) * P],
                                scalar1=rq4[:, h:h + 1])
                        stp = ptp.tile([P, H, P], f32, name=tag + "_stp",
                                       tag="tp")
                        for h in range(H):
                            nc.tensor.matmul(stp[:, h, :],
                                             lhsT=seq[:, h * P:(h + 1) * P],
                                             rhs=id_bf, start=True, stop=True)
                        seqT = a16.tile([P, H, P], bf16, name=tag + "_seqT",
                                        tag="seqT16", bufs=3)
                        nc.vector.tensor_copy(out=seqT, in_=stp)
                        att = pmm.tile([P, FD], f32, name=tag + "_att",
                                       tag="mm")
                        for h in range(H):
                            nc.tensor.matmul(att, lhsT=seqT[:, h, :],
                                             rhs=M_bf[h], start=(h == 0),
                                             stop=(h == H - 1))
                        rt = load_nt(dr[resid], t, tag + "_rt")
                        ln_tile(tag, att, rt, br, final, zn_dr, zT_dr, zTc, t)

        # ---------------- MLP ----------------
        def mlp(tag, zinT, resid, zn_dr, zT_dr, final=False, b2=None):
            with tc.spectator_scope(tag):
                zTc = None
                for cch in range(NCH):
                    zin = load_ct_chunk(dr[zinT], cch, tag + "_zin")
                    if not final:
                        zTc = new_zTc(tag)
                    us = []
                    for ht in range(HT):
                        f1 = pmm.tile([P, FD], f32, name=tag + "_f1", tag="mm")
                        for i in range(CT):
                            nc.tensor.matmul(
                                f1, lhsT=w1sb[:, i, ht * P:(ht + 1) * P],
                                rhs=zin[:, i, :], start=(i == 0),
                                stop=(i == CT - 1))
                        ut = a16.tile([P, FD], bf16, name=tag + "_u",
                                      tag="u16", bufs=20)
                        if b1c is not None:
                            nc.scalar.activation(out=ut, in_=f1, func=AF.Relu,
                                                 bias=b1c[:, ht:ht + 1])
                        else:
                            nc.scalar.activation(out=ut, in_=f1, func=AF.Relu)
                        us.append(ut)
                    for tsub in range(4):
                        t = 4 * cch + tsub
                        f2 = pmm.tile([P, FD], f32, name=tag + "_f2", tag="mm")
                        for ht in range(HT):
                            nc.tensor.matmul(
                                f2, lhsT=us[ht][:, tsub * P:(tsub + 1) * P],
                                rhs=w2sb[:, ht, :], start=(ht == 0),
                                stop=(ht == HT - 1))
                        rt = load_nt(dr[resid], t, tag + "_rt")
                        ln_tile(tag, f2, rt, b2, final, zn_dr, zT_dr, zTc, t)

        bb = lambda n: bcast.get(n)

        eattn("xsa", "xT_bf", "xT_bf", "sa", "pk_sa_x", "pq_sa_x", "x_nt",
              dr["zn_osa"], dr["zT_osa"], bv=bb("sa_v"), br=bb("sa_r"))
        eattn("ysa", "yT_bf", "yT_bf", "sa", "pk_sa_y", "pq_sa_y", "y_nt",
              dr["zn_ysa"], dr["zT_ysa"], bv=bb("sa_v"), br=bb("sa_r"))
        eattn("xca", "zT_osa", "qT_bf", "ca", "pk_ca_x", "pq_ca_x", "zn_osa",
              dr["zn_oca"], dr["zT_oca"], bv=bb("ca_v"), br=bb("ca_r"))
        mlp("xml", "zT_oca", "zn_oca", None, dr["zT_oo"], b2=bb("mlp2"))
        eattn("yca", "zT_ysa", "zT_oo", "ca", "pk_ca_y", "pq_ca_y", "zn_ysa",
              dr["zn_yca"], dr["zT_yca"], bv=bb("ca_v"), br=bb("ca_r"))
        mlp("yml", "zT_yca", "zn_yca", None, None, final=True, b2=bb("mlp2"))

    nc.compile()
    return nc


# ======================= host side =======================

_NC_CACHE = {}
LAST_RESULT = None


def _get_nc(N, ln_affine, biases):
    key = (N, ln_affine, tuple(sorted(biases)))
    if key not in _NC_CACHE:
        _NC_CACHE[key] = build_nc(N, ln_affine, frozenset(biases))
    return _NC_CACHE[key]


def _bf(a):
    return np.ascontiguousarray(a.astype(ml_dtypes.bfloat16))


def host_prep(inputs, N):
    """Common (core-independent) in_map entries."""
    ws = {w: np.asarray(inputs[w + "_w"], np.float32) for w in ATTN_W}
    bs = {w: np.asarray(inputs[w + "_b"], np.float32) for w in ATTN_W}
    posx = np.asarray(inputs["pos_x"], np.float32)[0]  # (N, C)
    posy = np.asarray(inputs["pos_y"], np.float32)[0]
    m = {}
    for w in ATTN_W:
        m[w + "_w"] = _bf(ws[w])
    m["mlp_w1"] = _bf(np.asarray(inputs["mlp_w1"], np.float32))
    m["mlp_w2"] = _bf(np.asarray(inputs["mlp_w2"], np.float32))
    # pos projections with q/k biases folded in (token-major)
    m["pq_sa_x"] = _bf(posx @ ws["sa_q"] + bs["sa_q"])
    m["pq_ca_x"] = _bf(posx @ ws["ca_q"] + bs["ca_q"])
    m["pq_sa_y"] = _bf(posy @ ws["sa_q"] + bs["sa_q"])
    m["pq_ca_y"] = _bf(posy @ ws["ca_q"] + bs["ca_q"])
    m["pk_sa_x"] = _bf(posx @ ws["sa_k"] + bs["sa_k"])
    m["pk_ca_x"] = _bf(posx @ ws["ca_k"] + bs["ca_k"])
    m["pk_sa_y"] = _bf(posy @ ws["sa_k"] + bs["sa_k"])
    m["pk_ca_y"] = _bf(posy @ ws["ca_k"] + bs["ca_k"])
    bias_arr = {"sa_v": "sa_v_b", "sa_r": "sa_r_b",
                "ca_v": "ca_v_b", "ca_r": "ca_r_b",
                "mlp1": "mlp_b1", "mlp2": "mlp_b2"}
    biases = set()
    for bn, an in bias_arr.items():
        arr = np.asarray(inputs[an], np.float32)
        if np.any(arr != 0):
            biases.add(bn)
            if bn == "mlp1":
                m["b_mlp1"] = np.ascontiguousarray(arr.reshape(-1))
            else:
                m["b_" + bn] = _bf(arr.reshape(1, -1))
    g = np.asarray(inputs["ln_g"], np.float32)
    b = np.asarray(inputs["ln_b"], np.float32)
    ln_affine = bool(np.any(g != 1) or np.any(b != 0))
    if ln_affine:
        m["ln_g"] = np.ascontiguousarray(g.reshape(1, -1))
        m["ln_b"] = np.ascontiguousarray(b.reshape(1, -1))
    return m, biases, ln_affine


def core_inputs(inputs, b):
    x = np.asarray(inputs["x"], np.float32)[b]
    y = np.asarray(inputs["y"], np.float32)[b]
    q = np.asarray(inputs["q"], np.float32)[b]
    return {"xT_bf": _bf(x.T), "yT_bf": _bf(y.T), "qT_bf": _bf(q.T),
            "x_nt": _bf(x), "y_nt": _bf(y)}


def kernel(**inputs):
    from concourse import bass_utils
    N = np.asarray(inputs["x"]).shape[1]
    B = np.asarray(inputs["x"]).shape[0]
    common, biases, ln_affine = host_prep(inputs, N)
    nc = _get_nc(N, ln_affine, biases)
    in_maps = []
    for b in range(B):
        m = dict(common)
        m.update(core_inputs(inputs, b))
        in_maps.append(m)
    res = bass_utils.run_bass_kernel_spmd(nc, in_maps, core_ids=list(range(B)))
    global LAST_RESULT
    LAST_RESULT = res
    out = np.stack([r["yO"] for r in res.results], axis=0)
    return np.ascontiguousarray(out.astype(np.float32))


# revision 12
# speedup vs baseline: 1.3049x; 1.0295x over previous
"""Trainium2 Bass kernel for nn_Block_34256659153605 (dual-branch linear-attention
transformer block). Data-parallel over batch B=8 across 8 NeuronCores; each core
runs the full block for one batch item.

Layout strategy (v2, token-major):
  - Projections produce NT tiles (128 tokens on partitions, channels free), so
    every per-token reduction (q-softmax denominators, LayerNorm mean/var) is a
    partition-parallel free-axis accumulate (ACT/DVE accum_out), with small
    [128,H] reciprocals -- no single-partition DVE ops, no DRAM stat round
    trips, no partition broadcasts.
  - The k-softmax (over tokens) stays a PE ones-matmul; its per-channel
    reciprocal is folded into M_h = (1/s_k) * ctx_h @ Wr_h, which also folds
    the context matrix into the reprojection (no separate att matmul).
  - Next layer's contractions need channel-major (CT) operands, produced by
    PE transposes (128-col matmuls against identity) of the LN output tiles.
  - Residual adds and positional-projection adds ride the PSUM evictions on
    DVE (scalar_tensor_tensor) instead of PE identity-matmuls.
  - pos embeddings folded on host: (x+pos)@W = x@W + (pos@W + b), the latter
    precomputed in numpy (q/k biases folded there too).
"""

import os
import sys
import numpy as np

if "/opt/trn_rl_repo" not in sys.path:
    sys.path.insert(0, "/opt/trn_rl_repo")

import ml_dtypes
from contextlib import ExitStack

import concourse.bass as bass
import concourse.mybir as mybir
import concourse.tile as tile
from concourse import bacc
from concourse.masks import make_identity

P = 128
C = 512
H = 4
HID = 4 * C
CT = C // P          # 4 channel blocks
HT = HID // P        # 16 hidden blocks
FD = 512             # token chunk size
EPS = 1e-5

bf16 = mybir.dt.bfloat16
f32 = mybir.dt.float32
AF = mybir.ActivationFunctionType
ALU = mybir.AluOpType

ATTN_W = ["sa_q", "sa_k", "sa_v", "sa_r", "ca_q", "ca_k", "ca_v", "ca_r"]


def build_nc(N=2048, ln_affine=False, biases=frozenset()):
    NCH = N // FD
    NTL = N // P  # token tiles
    nc = bacc.Bacc("TRN2", debug=False)

    dr = {}
    def din(name, shape, dt, kind="ExternalInput"):
        dr[name] = nc.dram_tensor(name, shape, dt, kind=kind).ap()

    din("xT_bf", (C, N), bf16)
    din("yT_bf", (C, N), bf16)
    din("qT_bf", (C, N), bf16)
    din("x_nt", (N, C), bf16)
    din("y_nt", (N, C), bf16)
    for w in ATTN_W:
        din(w + "_w", (C, C), bf16)
    din("mlp_w1", (C, HID), bf16)
    din("mlp_w2", (HID, C), bf16)
    # pos projections (+ folded q/k biases), token-major (N, C)
    for nm in ["pq_sa_x", "pq_ca_x", "pq_sa_y", "pq_ca_y",
               "pk_sa_x", "pk_ca_x", "pk_sa_y", "pk_ca_y"]:
        din(nm, (N, C), bf16)
    for bn in biases:
        if bn == "mlp1":
            din("b_mlp1", (HID,), f32)
        else:
            din("b_" + bn, (1, C), bf16)
    if ln_affine:
        din("ln_g", (1, C), f32)
        din("ln_b", (1, C), f32)
    # inter-layer handoffs
    for nm in ["zT_osa", "zT_oca", "zT_oo", "zT_ysa", "zT_yca"]:
        din(nm, (C, N), bf16, kind="Internal")
    for nm in ["zn_osa", "zn_oca", "zn_ysa", "zn_yca"]:
        din(nm, (N, C), bf16, kind="Internal")
    out_d = nc.dram_tensor("yO", (N, C), f32, kind="ExternalOutput").ap()

    ctv = lambda d: d.rearrange("(i p) n -> p i n", p=P)   # chunk loads
    ctw = lambda d: d.rearrange("(i p) n -> p i n", p=P)   # zT writes
    ntv = lambda d: d.rearrange("(t p) c -> t p c", p=P)   # NT tiles

    with tile.TileContext(nc) as tc, ExitStack() as ctx:
        consts = ctx.enter_context(tc.tile_pool(name="consts", bufs=1))
        a16 = ctx.enter_context(tc.tile_pool(name="a16", bufs=2))
        a32 = ctx.enter_context(tc.tile_pool(name="a32", bufs=2))
        pmm = ctx.enter_context(tc.tile_pool(name="pmm", bufs=6, space="PSUM"))
        psm = ctx.enter_context(tc.tile_pool(name="psm", bufs=2, space="PSUM"))

        # ---------------- persistent constants ----------------
        def wload(name, dram, nblk, fd):
            t = consts.tile([P, nblk, fd], bf16, name=name)
            nc.sync.dma_start(out=t, in_=dram.rearrange("(i p) c -> p i c", p=P))
            return t

        wsb = {w: wload("w_" + w, dr[w + "_w"], CT, C) for w in ATTN_W}
        w1sb = wload("w_mlp1", dr["mlp_w1"], CT, HID)
        w2sb = wload("w_mlp2", dr["mlp_w2"], HT, C)

        id_bf = consts.tile([P, P], bf16, name="id_bf")
        make_identity(nc, id_bf)
        ones_bf = consts.tile([P, 1], bf16, name="ones_bf")
        nc.vector.memset(ones_bf, 1.0)
        ones_f = consts.tile([P, 1], f32, name="ones_f")
        nc.vector.memset(ones_f, 1.0)
        eps_t = consts.tile([P, 1], f32, name="eps_t")
        nc.vector.memset(eps_t, EPS)

        # bias / affine constants (only when active; zero in practice)
        bcast = {}

        def dram_bcast_row(a):
            """DRAM AP (1, F) -> broadcast AP (128, F)."""
            return bass.AP(tensor=a.tensor, offset=a.offset,
                           ap=[[0, P]] + [list(d) for d in a.ap[1:]])

        def row_bcast(name, src_row, dt):
            bt = consts.tile([P, C], dt, name=name + "_bc")
            nc.sync.dma_start(out=bt, in_=dram_bcast_row(src_row))
            return bt

        for bn in ("sa_v", "ca_v", "sa_r", "ca_r", "mlp2"):
            if bn in biases:
                bcast[bn] = row_bcast("b" + bn, dr["b_" + bn], bf16)
        b1c = None
        if "mlp1" in biases:
            b1c = consts.tile([P, HT], f32, name="b1c")
            nc.sync.dma_start(out=b1c,
                              in_=dr["b_mlp1"].rearrange("(i p) -> p i", p=P))
        g_bc = b_bc = None
        if ln_affine:
            g_bc = row_bcast("lng", dr["ln_g"], f32)
            b_bc = row_bcast("lnb", dr["ln_b"], f32)

        # ---------------- helpers ----------------
        def load_ct_chunk(d, c, name):
            tl = a16.tile([P, CT, FD], bf16, name=name, tag="ld16", bufs=3)
            nc.sync.dma_start(out=tl, in_=ctv(d)[:, :, c * FD:(c + 1) * FD])
            return tl

        def load_nt(d, t, name, tag="nt16", bufs=6):
            tl = a16.tile([P, C], bf16, name=name, tag=tag, bufs=bufs)
            nc.sync.dma_start(out=tl, in_=ntv(d)[t])
            return tl

        def ln_tile(tag, ps, rt, badd, sc, hs, t):
            """PSUM (tok, chan) + resid -> h (f32) + accumulate LN stats."""
            tt = t % 4
            h_ = a32.tile([P, FD], f32, name=tag + "_h", tag="h32", bufs=6)
            nc.vector.scalar_tensor_tensor(
                out=h_, in0=ps, scalar=1.0, in1=rt,
                op0=ALU.mult, op1=ALU.add, accum_out=sc[:, tt, 0:1])
            if badd is not None:
                nc.vector.tensor_tensor(out=h_, in0=h_, in1=badd, op=ALU.add)
            hsq = a16.tile([P, FD], bf16, name=tag + "_hsq", tag="sq16", bufs=2)
            nc.scalar.activation(out=hsq, in_=h_, func=AF.Square,
                                 accum_out=sc[:, tt, 1:2])
            hs.append(h_)

        def ln_chunk(tag, sc, hs, final, zn_dr, zT_dr, zTc, cch):
            """Batched LN column math + apply + transpose for one chunk."""
            mq = a32.tile([P, 4, 2], f32, name=tag + "_mq", tag="sc", bufs=3)
            nc.vector.tensor_scalar_mul(out=mq, in0=sc, scalar1=1.0 / C)
            m2 = a32.tile([P, 4, 2], f32, name=tag + "_m2", tag="sc", bufs=3)
            nc.vector.tensor_tensor(out=m2[:, :, 0], in0=mq[:, :, 0],
                                    in1=mq[:, :, 0], op=ALU.mult)
            nc.vector.tensor_tensor(out=m2[:, :, 1], in0=mq[:, :, 1],
                                    in1=m2[:, :, 0], op=ALU.subtract)
            # rstd = exp(-0.5*ln(var+eps)): Ln/Exp live in one ACT table set
            # (natural_log_exp_and_others) together with Square/Relu/Copy, so
            # the whole kernel runs with zero activation-table switches.
            lnv = a32.tile([P, 4], f32, name=tag + "_lnv", tag="st4", bufs=3)
            nc.scalar.activation(out=lnv, in_=m2[:, :, 1], func=AF.Ln,
                                 bias=eps_t[:, 0:1])
            rstd = a32.tile([P, 4], f32, name=tag + "_rstd", tag="st4", bufs=3)
            nc.scalar.activation(out=rstd, in_=lnv, func=AF.Exp, scale=-0.5)
            for tt in range(4):
                t = 4 * cch + tt
                h_ = hs[tt]
                if final:
                    zf = a32.tile([P, FD], f32, name=tag + "_zf", tag="zf32",
                                  bufs=3)
                    nc.vector.tensor_scalar(out=zf, in0=h_,
                                            scalar1=mq[:, tt, 0:1],
                                            scalar2=rstd[:, tt:tt + 1],
                                            op0=ALU.subtract, op1=ALU.mult)
                    nc.sync.dma_start(out=ntv(out_d)[t], in_=zf)
                    continue
                if ln_affine:
                    z0 = a32.tile([P, FD], f32, name=tag + "_z0", tag="zf32",
                                  bufs=3)
                    nc.vector.tensor_scalar(out=z0, in0=h_,
                                            scalar1=mq[:, tt, 0:1],
                                            scalar2=rstd[:, tt:tt + 1],
                                            op0=ALU.subtract, op1=ALU.mult)
                    z1 = a32.tile([P, FD], f32, name=tag + "_z1", tag="zf32",
                                  bufs=3)
                    nc.vector.tensor_tensor(out=z1, in0=z0, in1=g_bc,
                                            op=ALU.mult)
                    zb = a16.tile([P, FD], bf16, name=tag + "_zb", tag="zb16",
                                  bufs=4)
                    nc.vector.tensor_tensor(out=zb, in0=z1, in1=b_bc,
                                            op=ALU.add)
                else:
                    zb = a16.tile([P, FD], bf16, name=tag + "_zb", tag="zb16",
                                  bufs=4)
                    nc.vector.tensor_scalar(out=zb, in0=h_,
                                            scalar1=mq[:, tt, 0:1],
                                            scalar2=rstd[:, tt:tt + 1],
                                            op0=ALU.subtract, op1=ALU.mult)
                if zn_dr is not None:
                    nc.sync.dma_start(out=ntv(zn_dr)[t], in_=zb)
                tp = pmm.tile([P, CT, P], f32, name=tag + "_ztp", tag="mm")
                for i in range(CT):
                    nc.tensor.matmul(tp[:, i, :], lhsT=zb[:, i * P:(i + 1) * P],
                                     rhs=id_bf, start=True, stop=True)
                nc.vector.tensor_copy(out=zTc[:, :, tt * P:(tt + 1) * P],
                                      in_=tp)
            if not final:
                nc.sync.dma_start(
                    out=ctw(zT_dr)[:, :, cch * FD:(cch + 1) * FD], in_=zTc)

        def new_zTc(tag):
            return a16.tile([P, CT, FD], bf16, name=tag + "_zTc",
                            tag="zTc", bufs=2)

        # ---------------- efficient attention ----------------
        def eattn(tag, kinT, qinT, W, pk, pq, resid, zn_dr, zT_dr,
                  final=False, bv=None, br=None):
            wq, wk, wv, wr = (wsb[W + "_q"], wsb[W + "_k"],
                              wsb[W + "_v"], wsb[W + "_r"])
            with tc.spectator_scope(tag):
                # ---- phase 1: kp/vp/ek/vt, sk (ones-MM), ctx^T ----
                sk_ps = psm.tile([P, FD], f32, name=tag + "_sk", tag="sm")
                ctxT_ps = psm.tile([P, H, P], f32, name=tag + "_ctxT", tag="sm")
                for cch in range(NCH):
                    kin = load_ct_chunk(dr[kinT], cch, tag + "_kin")
                    for tsub in range(4):
                        t = 4 * cch + tsub
                        kp = pmm.tile([P, FD], f32, name=tag + "_kp", tag="mm")
                        for i in range(CT):
                            nc.tensor.matmul(
                                kp, lhsT=kin[:, i, tsub * P:(tsub + 1) * P],
                                rhs=wk[:, i, :], start=(i == 0),
                                stop=(i == CT - 1))
                        pkt = load_nt(dr[pk], t, tag + "_pkt")
                        kadd = a16.tile([P, FD], bf16, name=tag + "_kadd",
                                        tag="add16", bufs=3)
                        nc.vector.scalar_tensor_tensor(
                            out=kadd, in0=kp, scalar=1.0, in1=pkt,
                            op0=ALU.mult, op1=ALU.add)
                        ek = a16.tile([P, FD], bf16, name=tag + "_ek",
                                      tag="ek16", bufs=3)
                        nc.scalar.activation(out=ek, in_=kadd, func=AF.Exp)
                        vp = pmm.tile([P, FD], f32, name=tag + "_vp", tag="mm")
                        for i in range(CT):
                            nc.tensor.matmul(
                                vp, lhsT=kin[:, i, tsub * P:(tsub + 1) * P],
                                rhs=wv[:, i, :], start=(i == 0),
                                stop=(i == CT - 1))
                        vt = a16.tile([P, FD], bf16, name=tag + "_vt",
                                      tag="vt16", bufs=3)
                        if bv is not None:
                            nc.vector.tensor_tensor(out=vt, in0=vp, in1=bv,
                                                    op=ALU.add)
                        else:
                            nc.scalar.activation(out=vt, in_=vp, func=AF.Copy)
                        nc.tensor.matmul(sk_ps[0:1, :], lhsT=ones_bf, rhs=ek,
                                         start=(t == 0), stop=(t == NTL - 1))
                        for h in range(H):
                            nc.tensor.matmul(
                                ctxT_ps[:, h, :],
                                lhsT=vt[:, h * P:(h + 1) * P],
                                rhs=ek[:, h * P:(h + 1) * P],
                                start=(t == 0), stop=(t == NTL - 1))
                # ---- phase 2: rk = 1/s_k ; M_h = rk * ctx_h @ Wr_h ----
                skrow = a32.tile([1, FD], f32, name=tag + "_skrow",
                                 tag="skrow", bufs=2)
                nc.vector.tensor_copy(out=skrow, in_=sk_ps[0:1, :])
                ktp = pmm.tile([P, H], f32, name=tag + "_ktp", tag="mm")
                for h in range(H):
                    nc.tensor.transpose(ktp[:, h:h + 1],
                                        skrow[0:1, h * P:(h + 1) * P],
                                        ones_f[0:1, 0:1])
                rk = a32.tile([P, H], f32, name=tag + "_rk", tag="rk", bufs=4)
                nc.vector.reciprocal(out=rk, in_=ktp)
                ctxT_bf = a16.tile([P, H, P], bf16, name=tag + "_ctxbf",
                                   tag="ctx16", bufs=2)
                nc.vector.tensor_copy(out=ctxT_bf, in_=ctxT_ps)
                M_bf = []
                for h in range(H):
                    mps = pmm.tile([P, FD], f32, name=tag + "_mps", tag="mm")
                    nc.tensor.matmul(mps, lhsT=ctxT_bf[:, h, :],
                                     rhs=wr[:, h, :], start=True, stop=True)
                    mb = a16.tile([P, FD], bf16, name=tag + "_mbf",
                                  tag="M16", bufs=5)
                    nc.vector.tensor_scalar_mul(out=mb, in0=mps,
                                                scalar1=rk[:, h:h + 1])
                    M_bf.append(mb)
                # ---- phase 3: qp -> softmax -> seq^T -> out_nt -> LN ----
                zTc = None
                for cch in range(NCH):
                    qin = load_ct_chunk(dr[qinT], cch, tag + "_qin")
                    if not final:
                        zTc = new_zTc(tag)
                    sc = a32.tile([P, 4, 2], f32, name=tag + "_sc", tag="sc",
                                  bufs=3)
                    hs = []
                    for tsub in range(4):
                        t = 4 * cch + tsub
                        qp = pmm.tile([P, FD], f32, name=tag + "_qp", tag="mm")
                        for i in range(CT):
                            nc.tensor.matmul(
                                qp, lhsT=qin[:, i, tsub * P:(tsub + 1) * P],
                                rhs=wq[:, i, :], start=(i == 0),
                                stop=(i == CT - 1))
                        pqt = load_nt(dr[pq], t, tag + "_pqt")
                        qadd = a16.tile([P, FD], bf16, name=tag + "_qadd",
                                        tag="add16", bufs=3)
                        nc.vector.scalar_tensor_tensor(
                            out=qadd, in0=qp, scalar=1.0, in1=pqt,
                            op0=ALU.mult, op1=ALU.add)
                        eq = a16.tile([P, H, P], bf16, name=tag + "_eq",
                                      tag="eq16", bufs=3)
                        nc.scalar.activation(out=eq, in_=qadd, func=AF.Exp)
                        sq4 = a32.tile([P, H], f32, name=tag + "_sq4",
                                       tag="rk", bufs=4)
                        nc.vector.tensor_reduce(out=sq4, in_=eq,
                                                axis=mybir.AxisListType.X,
                                                op=ALU.add)
                        rq4 = a32.tile([P, H], f32, name=tag + "_rq4",
                                       tag="rk", bufs=4)
                        nc.vector.reciprocal(out=rq4, in_=sq4)
                        seq = a16.tile([P, FD], bf16, name=tag + "_seq",
                                       tag="seq16", bufs=3)
                        for h in range(H):
                            nc.vector.tensor_scalar_mul(
                                out=seq[:, h * P:(h + 1) * P],
                                in0=eq[:, h, :],
                                scalar1=rq4[:, h:h + 1])
                        stp = pmm.tile([P, H, P], f32, name=tag + "_stp",
                                       tag="mm")
                        for h in range(H):
                            nc.tensor.matmul(stp[:, h, :],
                                             lhsT=seq[:, h * P:(h + 1) * P],
                                             rhs=id_bf, start=True, stop=True)
                        seqT = a16.tile([P, H, P], bf16, name=tag + "_seqT",
                                        tag="seqT16", bufs=3)
                        nc.vector.tensor_copy(out=seqT, in_=stp)
                        att = pmm.tile([P, FD], f32, name=tag + "_att",
                                       tag="mm")
                        for h in range(H):
                            nc.tensor.matmul(att, lhsT=seqT[:, h, :],
                                             rhs=M_bf[h], start=(h == 0),
                                             stop=(h == H - 1))
                        rt = load_nt(dr[resid], t, tag + "_rt")
                        ln_tile(tag, att, rt, br, sc, hs, t)
                    ln_chunk(tag, sc, hs, final, zn_dr, zT_dr, zTc, cch)

        # ---------------- MLP ----------------
        def mlp(tag, zinT, resid, zn_dr, zT_dr, final=False, b2=None):
            with tc.spectator_scope(tag):
                zTc = None
                for cch in range(NCH):
                    zin = load_ct_chunk(dr[zinT], cch, tag + "_zin")
                    if not final:
                        zTc = new_zTc(tag)
                    sc = a32.tile([P, 4, 2], f32, name=tag + "_sc", tag="sc",
                                  bufs=3)
                    hs = []
                    us = []
                    for ht in range(HT):
                        f1 = pmm.tile([P, FD], f32, name=tag + "_f1", tag="mm")
                        for i in range(CT):
                            nc.tensor.matmul(
                                f1, lhsT=w1sb[:, i, ht * P:(ht + 1) * P],
                                rhs=zin[:, i, :], start=(i == 0),
                                stop=(i == CT - 1))
                        ut = a16.tile([P, FD], bf16, name=tag + "_u",
                                      tag="u16", bufs=20)
                        if b1c is not None:
                            nc.scalar.activation(out=ut, in_=f1, func=AF.Relu,
                                                 bias=b1c[:, ht:ht + 1])
                        else:
                            nc.scalar.activation(out=ut, in_=f1, func=AF.Relu)
                        us.append(ut)
                    for tsub in range(4):
                        t = 4 * cch + tsub
                        f2 = pmm.tile([P, FD], f32, name=tag + "_f2", tag="mm")
                        for ht in range(HT):
                            nc.tensor.matmul(
                                f2, lhsT=us[ht][:, tsub * P:(tsub + 1) * P],
                                rhs=w2sb[:, ht, :], start=(ht == 0),
                                stop=(ht == HT - 1))
                        rt = load_nt(dr[resid], t, tag + "_rt")
                        ln_tile(tag, f2, rt, b2, sc, hs, t)
                    ln_chunk(tag, sc, hs, final, zn_dr, zT_dr, zTc, cch)

        bb = lambda n: bcast.get(n)

        eattn("xsa", "xT_bf", "xT_bf", "sa", "pk_sa_x", "pq_sa_x", "x_nt",
              dr["zn_osa"], dr["zT_osa"], bv=bb("sa_v"), br=bb("sa_r"))
        eattn("ysa", "yT_bf", "yT_bf", "sa", "pk_sa_y", "pq_sa_y", "y_nt",
              dr["zn_ysa"], dr["zT_ysa"], bv=bb("sa_v"), br=bb("sa_r"))
        eattn("xca", "zT_osa", "qT_bf", "ca", "pk_ca_x", "pq_ca_x", "zn_osa",
              dr["zn_oca"], dr["zT_oca"], bv=bb("ca_v"), br=bb("ca_r"))
        mlp("xml", "zT_oca", "zn_oca", None, dr["zT_oo"], b2=bb("mlp2"))
        eattn("yca", "zT_ysa", "zT_oo", "ca", "pk_ca_y", "pq_ca_y", "zn_ysa",
              dr["zn_yca"], dr["zT_yca"], bv=bb("ca_v"), br=bb("ca_r"))
        mlp("yml", "zT_yca", "zn_yca", None, None, final=True, b2=bb("mlp2"))

    nc.compile()
    return nc


# ======================= host side =======================

_NC_CACHE = {}
LAST_RESULT = None


def _get_nc(N, ln_affine, biases):
    key = (N, ln_affine, tuple(sorted(biases)))
    if key not in _NC_CACHE:
        _NC_CACHE[key] = build_nc(N, ln_affine, frozenset(biases))
    return _NC_CACHE[key]


def _bf(a):
    return np.ascontiguousarray(a.astype(ml_dtypes.bfloat16))


def host_prep(inputs, N):
    """Common (core-independent) in_map entries."""
    ws = {w: np.asarray(inputs[w + "_w"], np.float32) for w in ATTN_W}
    bs = {w: np.asarray(inputs[w + "_b"], np.float32) for w in ATTN_W}
    posx = np.asarray(inputs["pos_x"], np.float32)[0]  # (N, C)
    posy = np.asarray(inputs["pos_y"], np.float32)[0]
    m = {}
    for w in ATTN_W:
        m[w + "_w"] = _bf(ws[w])
    m["mlp_w1"] = _bf(np.asarray(inputs["mlp_w1"], np.float32))
    m["mlp_w2"] = _bf(np.asarray(inputs["mlp_w2"], np.float32))
    # pos projections with q/k biases folded in (token-major)
    m["pq_sa_x"] = _bf(posx @ ws["sa_q"] + bs["sa_q"])
    m["pq_ca_x"] = _bf(posx @ ws["ca_q"] + bs["ca_q"])
    m["pq_sa_y"] = _bf(posy @ ws["sa_q"] + bs["sa_q"])
    m["pq_ca_y"] = _bf(posy @ ws["ca_q"] + bs["ca_q"])
    m["pk_sa_x"] = _bf(posx @ ws["sa_k"] + bs["sa_k"])
    m["pk_ca_x"] = _bf(posx @ ws["ca_k"] + bs["ca_k"])
    m["pk_sa_y"] = _bf(posy @ ws["sa_k"] + bs["sa_k"])
    m["pk_ca_y"] = _bf(posy @ ws["ca_k"] + bs["ca_k"])
    bias_arr = {"sa_v": "sa_v_b", "sa_r": "sa_r_b",
                "ca_v": "ca_v_b", "ca_r": "ca_r_b",
                "mlp1": "mlp_b1", "mlp2": "mlp_b2"}
    biases = set()
    for bn, an in bias_arr.items():
        arr = np.asarray(inputs[an], np.float32)
        if np.any(arr != 0):
            biases.add(bn)
            if bn == "mlp1":
                m["b_mlp1"] = np.ascontiguousarray(arr.reshape(-1))
            else:
                m["b_" + bn] = _bf(arr.reshape(1, -1))
    g = np.asarray(inputs["ln_g"], np.float32)
    b = np.asarray(inputs["ln_b"], np.float32)
    ln_affine = bool(np.any(g != 1) or np.any(b != 0))
    if ln_affine:
        m["ln_g"] = np.ascontiguousarray(g.reshape(1, -1))
        m["ln_b"] = np.ascontiguousarray(b.reshape(1, -1))
    return m, biases, ln_affine


def core_inputs(inputs, b):
    x = np.asarray(inputs["x"], np.float32)[b]
    y = np.asarray(inputs["y"], np.float32)[b]
    q = np.asarray(inputs["q"], np.float32)[b]
    return {"xT_bf": _bf(x.T), "yT_bf": _bf(y.T), "qT_bf": _bf(q.T),
            "x_nt": _bf(x), "y_nt": _bf(y)}


def kernel(**inputs):
    from concourse import bass_utils
    N = np.asarray(inputs["x"]).shape[1]
    B = np.asarray(inputs["x"]).shape[0]
    common, biases, ln_affine = host_prep(inputs, N)
    nc = _get_nc(N, ln_affine, biases)
    in_maps = []
    for b in range(B):
        m = dict(common)
        m.update(core_inputs(inputs, b))
        in_maps.append(m)
    res = bass_utils.run_bass_kernel_spmd(nc, in_maps, core_ids=list(range(B)))
    global LAST_RESULT
    LAST_RESULT = res
    out = np.stack([r["yO"] for r in res.results], axis=0)
    return np.ascontiguousarray(out.astype(np.float32))
